# revision 12
# baseline (speedup 1.0000x reference)
"""Trainium2 Bass kernel for a cross-attention block with 3D-coordinate RoPE.

Module: q/k/v projections of x [B,Tq,D] against memory [B,Tk,D], 3D-coord
rotary embedding on q/k, softmax(q k^T / sqrt(Hd)) v, output projection.
B=2, Tq=1024, Tk=2048, D=1536, 16 heads x 96.

Sharding: 8 cores = (2 batches) x (4 head-groups of 4 heads). Each core
computes its heads end-to-end plus a partial output projection; the host
sums the 4 partials per batch. Biases bv/bo are folded in on the host
(attention rows sum to one), bq/bk are added on-device during PSUM
eviction.

Layout: feature-major ("transposed") on device. Scores are computed
transposed (S^T = k q^T) so the PV matmul needs no on-chip transposes;
softmax denominators come from a ones-column appended to v; the
per-query normalization is broadcast across partitions with a K=1
ones-vector matmul + full-lane approximate reciprocal.

Schedule notes (v2):
 - PE warm-up matmuls run during the initial DMA flight so the HAM clock
   gate reaches 8/8 before the Q projection starts.
 - Input DMAs are spread across three issue queues (sync: wq/x, gpsimd:
   memory chunks, scalar: weights/tables) so the Q-phase dependency chain
   is not head-of-line blocked behind the K/V prefetch.
 - The K projection is chunk-outer (contraction-major) over column
   halves, consuming memory chunks in DMA arrival order.
 - RoPE uses an SBUF->SBUF DMA partition swap instead of a PE
   permutation matmul; the multiply/add chain runs on DVE in fp16.
 - Output is written fp16 (host accumulates partials in fp32).
"""

import os
import sys

sys.path.insert(0, "/opt/trn_rl_repo")

import numpy as np
import ml_dtypes
from contextlib import ExitStack

import concourse.bass as bass
import concourse.tile as tile
from concourse import bacc, mybir
from concourse.bass_utils import run_bass_kernel_spmd

# ---------------------------------------------------------------- constants
B = 2
TQ = 1024
TK = 2048
D = 1536
NH = 16
HD = 96
ROPE_HALF = HD // 2           # 48
FREQ_PER_AXIS = ROPE_HALF // 3  # 16
ROPE_BASE = 10000.0
NH_CORE = 4                   # heads per core
HG = NH_CORE * HD             # 384 features per core
KC = D // 128                 # 12 contraction chunks
MTILES = D // 128             # 12 output-row tiles of the o-projection
SCALE = 1.0 / float(np.sqrt(HD))
N_CORES = 8
VW = HD + 1                   # 97: head-dim + ones column

F32 = mybir.dt.float32

_MM_DT_NAME = os.environ.get("KMM_DTYPE", "f16")
_DT = {"bf16": mybir.dt.bfloat16, "f16": mybir.dt.float16}
_NP = {"bf16": ml_dtypes.bfloat16, "f16": np.float16}
QK_DT = PV_DT = _DT[_MM_DT_NAME]
QK_NP = PV_NP = _NP[_MM_DT_NAME]


# ---------------------------------------------------------------- bass build
def _build_nc():
    nc = bacc.Bacc(trn_type="TRN2", target_bir_lowering=False, debug=False)

    io = {}
    def dram_in(name, shape, dt):
        io[name] = nc.dram_tensor(name, list(shape), dt, kind="ExternalInput").ap()
    dram_in("xT", [D, TQ], QK_DT)
    dram_in("memT", [D, TK], QK_DT)
    dram_in("wqT", [D, HG], QK_DT)  # columns of Wq^T for this head group
    dram_in("wkT", [D, HG], QK_DT)
    dram_in("wvT", [D, HG], PV_DT)
    dram_in("woT", [HG, D], PV_DT)  # rows of Wo^T for this head group
    dram_in("bq4", [HD, NH_CORE], F32)
    dram_in("bk4", [HD, NH_CORE], F32)
    dram_in("cqE", [HD, TQ], QK_DT)  # cos table, feature-major, q side
    dram_in("sqE", [HD, TQ], QK_DT)  # sign-folded sin table, q side
    dram_in("ckE", [HD, TK], QK_DT)
    dram_in("skE", [HD, TK], QK_DT)
    oT = nc.dram_tensor("oT", [D, TQ], PV_DT, kind="ExternalOutput").ap()

    with tile.TileContext(nc) as tc, ExitStack() as ctx:
        _body(ctx, tc, io, oT)
    nc.compile()
    return nc


def _body(ctx, tc, io, oT):
    nc = tc.nc
    P = 128
    NKC = TK // P
    Exp = mybir.ActivationFunctionType.Exp
    Ident = mybir.ActivationFunctionType.Identity

    const = ctx.enter_context(tc.tile_pool(name="const", bufs=1))
    resident = ctx.enter_context(tc.tile_pool(name="resident", bufs=1))

    ones1_t = const.tile([1, P], PV_DT, name="ones1_t")
    bq_t = const.tile([HD, NH_CORE], F32, name="bq_t")
    bk_t = const.tile([HD, NH_CORE], F32, name="bk_t")
    cq_t = const.tile([HD, TQ], QK_DT, name="cq_t")
    sq_t = const.tile([HD, TQ], QK_DT, name="sq_t")
    ck_t = const.tile([HD, TK], QK_DT, name="ck_t")
    sk_t = const.tile([HD, TK], QK_DT, name="sk_t")
    wk_all = const.tile([P, KC, HG], QK_DT, name="wk_all")
    wv_all = const.tile([P, KC, HG], PV_DT, name="wv_all")
    qT = [resident.tile([HD, TQ], QK_DT, name=f"qT{h}", tag=f"qT{h}")
          for h in range(NH_CORE)]
    kT = [resident.tile([HD, TK], QK_DT, name=f"kT{h}", tag=f"kT{h}")
          for h in range(NH_CORE)]
    vst = [resident.tile([P, NH_CORE * VW], PV_DT, name=f"vst{m}", tag=f"vst{m}")
           for m in range(NKC)]
    tmp_pool = ctx.enter_context(tc.tile_pool(name="tmp_pool", bufs=4))
    p_pool = ctx.enter_context(tc.tile_pool(name="p_pool", bufs=10))
    # full memory^T resident: loaded once, serves K and V
    mem_stack = ExitStack()
    mem_pool = mem_stack.enter_context(tc.tile_pool(name="mem_pool", bufs=1))
    memR = [mem_pool.tile([P, TK], QK_DT, name=f"memR{c}", tag=f"memR{c}")
            for c in range(KC)]
    # swap-source tiles for RoPE, freed after the rope phase (opened after
    # mem_pool: pools close LIFO and sw closes first)
    sw_stack = ExitStack()
    sw_pool = sw_stack.enter_context(tc.tile_pool(name="sw_pool", bufs=1))
    qsw = [sw_pool.tile([HD, TQ], QK_DT, name=f"qsw{h}", tag=f"qsw{h}")
           for h in range(NH_CORE)]
    # ksw is half-width: reused for the second K column-half (WAR dep on
    # the first half's rope multiply keeps it correct)
    ksw = [sw_pool.tile([HD, TK // 2], QK_DT, name=f"ksw{h}", tag=f"ksw{h}")
           for h in range(NH_CORE)]

    # ---- PE warm-up: matmuls on memset tiles while input DMAs fly ---------
    with ExitStack() as wctx:
        warm_sb = wctx.enter_context(tc.tile_pool(name="warm_sb", bufs=1))
        warm_ps = wctx.enter_context(
            tc.tile_pool(name="warm_ps", bufs=1, space="PSUM"))
        wz = warm_sb.tile([P, 512], QK_DT, name="wz")
        nc.gpsimd.memset(wz[:], 0.0)
        wps = warm_ps.tile([P, 512], F32, name="wps", tag="wps")
        for _ in range(10):
            nc.tensor.matmul(wps[:], wz[:, 0:P], wz[:], start=True, stop=True)

    # ones for the denominator broadcast + v ones-columns (no DMA needed)
    nc.gpsimd.memset(ones1_t[:], 1.0)
    for m in range(NKC):
        ones_cols = vst[m].rearrange("p (h c) -> p h c", c=VW)[:, :, HD:HD + 1]
        nc.gpsimd.memset(ones_cols, 1.0)

    # ---- input DMA issue: three queues --------------------------------
    # gpsimd queue: memory chunks (feed K chunk-outer, then V)
    for c in range(KC):
        nc.gpsimd.dma_start(out=memR[c][:],
                            in_=io["memT"][c * P:(c + 1) * P, :])
    # scalar queue: weights + tables in consumption order
    nc.scalar.dma_start(out=bq_t[:], in_=io["bq4"][:])
    nc.scalar.dma_start(
        out=wk_all[:],
        in_=io["wkT"][:].rearrange("(c p) h -> p c h", p=P))
    nc.scalar.dma_start(out=cq_t[:], in_=io["cqE"][:])
    nc.scalar.dma_start(out=sq_t[:], in_=io["sqE"][:])
    nc.scalar.dma_start(out=bk_t[:], in_=io["bk4"][:])

    def rope(dst, swp, cE, sE, lo, width, swp_lo=None):
        """RoPE on dst[:, lo:lo+width] via DMA partition swap + DVE fp16.

        swp[j] = dst[(j+48) % 96]; dst <- dst*cE + swp*sE (sE sign-folded).
        """
        if swp_lo is None:
            swp_lo = lo
        sl = slice(lo, lo + width)
        ssl = slice(swp_lo, swp_lo + width)
        nc.sync.dma_start(out=swp[0:ROPE_HALF, ssl],
                          in_=dst[ROPE_HALF:HD, sl])
        nc.sync.dma_start(out=swp[ROPE_HALF:HD, ssl],
                          in_=dst[0:ROPE_HALF, sl])
        t1 = tmp_pool.tile([HD, width], QK_DT, name="t1", tag="tmp")
        t2 = tmp_pool.tile([HD, width], QK_DT, name="t2", tag="tmp")
        nc.vector.tensor_mul(t1[:], dst[:, sl], cE[:, sl])
        nc.vector.tensor_mul(t2[:], swp[:, ssl], sE[:, sl])
        nc.vector.tensor_add(dst[:, sl], t1[:], t2[:])

    # ---- phase Q: q^T = Wq_h @ x^T (c-outer, 8 psum banks) ---------------
    with ExitStack() as qctx:
        psq_pool = qctx.enter_context(
            tc.tile_pool(name="psq", bufs=NH_CORE, space="PSUM"))
        xq_pool = qctx.enter_context(tc.tile_pool(name="xq", bufs=4))
        wq_pool = qctx.enter_context(tc.tile_pool(name="wq", bufs=1))
        wq_all = wq_pool.tile([P, KC, HG], QK_DT, name="wq_all")
        psq = [psq_pool.tile([HD, TQ], F32, name=f"psq{h}", tag="psq")
               for h in range(NH_CORE)]
        for c in range(KC):
            nc.sync.dma_start(out=wq_all[:, c, :],
                              in_=io["wqT"][c * P:(c + 1) * P, :])
            xc = xq_pool.tile([P, TQ], QK_DT, name="xc", tag="xc")
            nc.sync.dma_start(out=xc[:], in_=io["xT"][c * P:(c + 1) * P, :])
            for h in range(NH_CORE):
                lhs = wq_all[:, c, h * HD:(h + 1) * HD]
                for n in range(2):
                    nc.tensor.matmul(
                        psq[h][:, n * 512:(n + 1) * 512],
                        lhs, xc[:, n * 512:(n + 1) * 512],
                        start=(c == 0), stop=(c == KC - 1))
        for h in range(NH_CORE):
            if h % 2 == 0:
                nc.vector.tensor_scalar_add(qT[h][:], psq[h][:],
                                            bq_t[:, h:h + 1])
            else:
                nc.scalar.activation(qT[h][:], psq[h][:], Ident,
                                     bias=bq_t[:, h:h + 1])

    # rope-q: swap DMAs on sync (free after Q loads), DVE muls overlap K
    for h in range(NH_CORE):
        rope(qT[h], qsw[h], cq_t, sq_t, 0, TQ)

    # k-side tables issue after the q tables (needed from ~mid-K)
    nc.scalar.dma_start(out=ck_t[:], in_=io["ckE"][:])
    nc.scalar.dma_start(out=sk_t[:], in_=io["skE"][:])

    # ---- phase K: k^T = Wk_h @ mem^T (chunk-outer over column halves) ----
    with ExitStack() as kctx:
        psk_pool = kctx.enter_context(
            tc.tile_pool(name="psk", bufs=8, space="PSUM"))
        for half in range(2):
            base = half * 1024
            psk = [psk_pool.tile([HD, 512], F32, name=f"psk{half}_{i}",
                                 tag="psk") for i in range(8)]
            for c in range(KC):
                for h in range(NH_CORE):
                    lhs = wk_all[:, c, h * HD:(h + 1) * HD]
                    for qq in range(2):
                        col = base + qq * 512
                        nc.tensor.matmul(
                            psk[qq * NH_CORE + h][:], lhs,
                            memR[c][:, col:col + 512],
                            start=(c == 0), stop=(c == KC - 1))
            for qq in range(2):
                col = base + qq * 512
                for h in range(NH_CORE):
                    if (h + qq) % 2 == 0:
                        nc.vector.tensor_scalar_add(
                            kT[h][:, col:col + 512],
                            psk[qq * NH_CORE + h][:], bk_t[:, h:h + 1])
                    else:
                        nc.scalar.activation(
                            kT[h][:, col:col + 512],
                            psk[qq * NH_CORE + h][:], Ident,
                            bias=bk_t[:, h:h + 1])
            for h in range(NH_CORE):
                rope(kT[h], ksw[h], ck_t, sk_t, base, 1024, swp_lo=0)
            if half == 0:
                nc.scalar.dma_start(
                    out=wv_all[:],
                    in_=io["wvT"][:].rearrange("(c p) h -> p c h", p=P))
    sw_stack.close()

    # attention score/exp pool opens before V so S/exp of head 0 can
    # overlap the V matmuls (ACT is otherwise idle during V)
    s_ps = ctx.enter_context(tc.tile_pool(name="s_ps", bufs=3, space="PSUM"))

    PTS = {}

    def emit_schunk(h, kc, _unused=None):
        st = s_ps.tile([P, TQ], F32, name="st", tag="s")
        lhs = kT[h][:, kc * P:(kc + 1) * P]
        nc.tensor.matmul(st[:, 0:512], lhs, qT[h][:, 0:512])
        nc.tensor.matmul(st[:, 512:1024], lhs, qT[h][:, 512:1024])
        pt = p_pool.tile([P, TQ], PV_DT, name="pt", tag="pt")
        nc.scalar.activation(pt[:], st[:], Exp, scale=SCALE)
        PTS[(h, kc)] = pt

    PRE = 16

    # ---- phase V: v natural [Tk, 4*97] (c-outer, by Tk m-pair) -----------
    with ExitStack() as vctx:
        psv_pool = vctx.enter_context(
            tc.tile_pool(name="psv", bufs=2, space="PSUM"))
        for q8 in range(8):
            col = q8 * 256
            psv = [psv_pool.tile([P, HG], F32, name=f"psv{q8}_{i}", tag="psv")
                   for i in range(2)]
            for c in range(KC):
                for ml in range(2):
                    nc.tensor.matmul(
                        psv[ml][:], memR[c][:, col + ml * P:col + (ml + 1) * P],
                        wv_all[:, c, :],
                        start=(c == 0), stop=(c == KC - 1))
            for ml in range(2):
                mg = q8 * 2 + ml
                dst = vst[mg].rearrange("p (h c) -> p h c", c=VW)[:, :, 0:HD]
                src = psv[ml].rearrange("p (h c) -> p h c", c=HD)
                nc.vector.tensor_copy(dst, src)
            if 1 <= q8 <= 7:     # pre-compute head 0 scores under V
                lo = [0, 0, 2, 4, 7, 10, 12, 14][q8]
                hi = [0, 2, 4, 7, 10, 12, 14, 16][q8]
                for k in range(lo, hi):
                    emit_schunk(0, k)
            if q8 == 0:
                wo_t = []
                for i in range(3):
                    w = const.tile([P, D], PV_DT, name=f"wo_t{i}",
                                   tag=f"wo_t{i}")
                    nc.scalar.dma_start(out=w[:],
                                        in_=io["woT"][i * P:(i + 1) * P, :])
                    wo_t.append(w)
    mem_stack.close()

    # ---- attention: one flat pipeline across all (head, chunk) pairs -----
    # PV lags S/exp by 3 chunks and flows straight across head boundaries,
    # so the ACT engine never drains between heads.
    pv_ps = ctx.enter_context(tc.tile_pool(name="pv_ps", bufs=2, space="PSUM"))
    aout_pool = ctx.enter_context(tc.tile_pool(name="aout_pool", bufs=2))
    aN_pool = ctx.enter_context(tc.tile_pool(name="aN_pool", bufs=1))
    ot_pool = ctx.enter_context(tc.tile_pool(name="ot_pool", bufs=3))

    # aoutN stacked as 3 tiles of 128 partitions (heads packed) so the
    # o-projection contracts in 3 chunks of 128 instead of 4 of 96
    aN = [aN_pool.tile([P, TQ], PV_DT, name=f"aN{i}", tag=f"aN{i}")
          for i in range(3)]
    # per-head write segments: (tile, tile_row0, head_row0, nrows)
    _SEG = {0: [(0, 0, 0, 96)],
            1: [(0, 96, 0, 32), (1, 0, 32, 32), (1, 32, 64, 32)],
            2: [(1, 64, 0, 64), (2, 0, 64, 32)],
            3: [(2, 32, 0, 32), (2, 64, 32, 32), (2, 96, 64, 32)]}

    pvs = {}

    def finish_head(h):
        pv0, pv1 = pvs.pop(h)
        aout = aout_pool.tile([VW, TQ], PV_DT, name="aout", tag="aout")
        # denominator row straight from PSUM so the broadcast matmul does
        # not serialize behind the aout eviction
        den1 = tmp_pool.tile([1, TQ], PV_DT, name="den1", tag="den1")
        nc.vector.tensor_copy(den1[:, 0:512], pv0[HD:HD + 1, :])
        nc.vector.tensor_copy(den1[:, 512:1024], pv1[HD:HD + 1, :])
        nc.vector.tensor_copy(aout[:, 0:512], pv0[:])
        nc.vector.tensor_copy(aout[:, 512:1024], pv1[:])
        denB = s_ps.tile([P, TQ], F32, name="denB", tag="s")
        for n in range(2):
            nc.tensor.matmul(denB[:, n * 512:(n + 1) * 512], ones1_t[:],
                             den1[:, n * 512:(n + 1) * 512])
        recB = tmp_pool.tile([HD, TQ], F32, name="recB", tag="tmp")
        nc.vector.reciprocal_approx_fast(out=recB[:], in_=denB[0:HD, :])
        for (ti, tr, hr, nr) in _SEG[h]:
            nc.vector.tensor_mul(aN[ti][tr:tr + nr, :],
                                 aout[hr:hr + nr, :], recB[hr:hr + nr, :])

    LAG = 3
    G = NH_CORE * NKC
    for g in range(G + LAG):
        if g < G:
            h, kc = divmod(g, NKC)
            if not (h == 0 and kc < PRE):
                emit_schunk(h, kc)
        if g >= LAG:
            h2, kc2 = divmod(g - LAG, NKC)
            if kc2 == 0:
                pvs[h2] = (
                    pv_ps.tile([VW, 512], F32, name=f"pv{h2}0", tag="pv"),
                    pv_ps.tile([VW, 512], F32, name=f"pv{h2}1", tag="pv"))
            pv0, pv1 = pvs[h2]
            pt = PTS[(h2, kc2)]
            vl = vst[kc2][:, h2 * VW:(h2 + 1) * VW]
            first, last = (kc2 == 0), (kc2 == NKC - 1)
            nc.tensor.matmul(pv0[:], vl, pt[:, 0:512], start=first, stop=last)
            nc.tensor.matmul(pv1[:], vl, pt[:, 512:1024],
                             start=first, stop=last)
            PTS.pop((h2, kc2))
            if last:
                finish_head(h2)

    # ---- output projection (h-inner accumulation, 2 matmuls per weight) --
    for m in range(MTILES):
        po0 = s_ps.tile([P, 512], F32, name="po0", tag="s")
        po1 = s_ps.tile([P, 512], F32, name="po1", tag="s")
        for i in range(3):
            lhs = wo_t[i][:, m * P:(m + 1) * P]
            nc.tensor.matmul(po0[:], lhs, aN[i][:, 0:512],
                             start=(i == 0), stop=(i == 2))
            nc.tensor.matmul(po1[:], lhs, aN[i][:, 512:1024],
                             start=(i == 0), stop=(i == 2))
        ot = ot_pool.tile([P, TQ], PV_DT, name="ot", tag="ot")
        nc.vector.tensor_copy(ot[:, 0:512], po0[:])
        nc.sync.dma_start(out=oT[m * P:(m + 1) * P, 0:512], in_=ot[:, 0:512])
        nc.scalar.copy(ot[:, 512:1024], po1[:])
        nc.sync.dma_start(out=oT[m * P:(m + 1) * P, 512:1024],
                          in_=ot[:, 512:1024])


# ---------------------------------------------------------------- host side
def _rope_tables(coords, T):
    """Feature-major cos/sin tables [HD, T] with the sign fold.

    Row j < 48 of the rotated output is q[j]*cos_j - q[j+48]*sin_j and row
    j >= 48 is q[j]*cos_{j-48} + q[j-48]*sin_{j-48}; the device computes
    rot = q * cE + swap(q) * sE with swap(q)[j] = q[(j+48) % 96].
    """
    coords = np.asarray(coords, np.float32)
    inv_freq = (1.0 / (ROPE_BASE ** (np.arange(FREQ_PER_AXIS, dtype=np.float32)
                                     / FREQ_PER_AXIS))).astype(np.float32)
    ang = coords[:, :, None] * inv_freq[None, None, :]   # [T, 3, 16]
    ang = ang.reshape(T, ROPE_HALF)                      # [T, 48]
    sin = np.sin(ang).astype(np.float32).T               # [48, T]
    cos = np.cos(ang).astype(np.float32).T
    cE = np.concatenate([cos, cos], axis=0)              # [96, T]
    sE = np.concatenate([-sin, sin], axis=0)
    return (np.ascontiguousarray(cE).astype(QK_NP),
            np.ascontiguousarray(sE).astype(QK_NP))


def _make_in_maps(inputs):
    x = np.asarray(inputs["x"], np.float32)
    memory = np.asarray(inputs["memory"], np.float32)
    qc = np.asarray(inputs["query_coords"], np.float32)
    mc = np.asarray(inputs["memory_coords"], np.float32)
    Wq = np.asarray(inputs["Wq"], np.float32)
    Wk = np.asarray(inputs["Wk"], np.float32)
    Wv = np.asarray(inputs["Wv"], np.float32)
    Wo = np.asarray(inputs["Wo"], np.float32)
    bq = np.asarray(inputs["bq"], np.float32)
    bk = np.asarray(inputs["bk"], np.float32)

    WqT = np.ascontiguousarray(Wq.T).astype(QK_NP)   # [in, out]
    WkT = np.ascontiguousarray(Wk.T).astype(QK_NP)
    WvT = np.ascontiguousarray(Wv.T).astype(PV_NP)
    WoT = np.ascontiguousarray(Wo.T).astype(PV_NP)

    per_batch = []
    for b in range(B):
        cqE, sqE = _rope_tables(qc[b], TQ)
        ckE, skE = _rope_tables(mc[b], TK)
        entry = {
            "xT": np.ascontiguousarray(x[b].T).astype(QK_NP),
            "memT": np.ascontiguousarray(memory[b].T).astype(QK_NP),
            "cqE": cqE, "sqE": sqE, "ckE": ckE, "skE": skE,
        }
        per_batch.append(entry)

    in_maps = []
    for core in range(N_CORES):
        b, g = divmod(core, NH_CORE)
        sl = slice(g * HG, (g + 1) * HG)
        m = dict(per_batch[b])
        m["wqT"] = np.ascontiguousarray(WqT[:, sl])
        m["wkT"] = np.ascontiguousarray(WkT[:, sl])
        m["wvT"] = np.ascontiguousarray(WvT[:, sl])
        m["woT"] = np.ascontiguousarray(WoT[sl, :])
        m["bq4"] = np.ascontiguousarray(bq[sl].reshape(NH_CORE, HD).T)
        m["bk4"] = np.ascontiguousarray(bk[sl].reshape(NH_CORE, HD).T)
        in_maps.append(m)
    return in_maps


def _assemble(results, inputs):
    Wo = np.asarray(inputs["Wo"], np.float32)
    bv = np.asarray(inputs["bv"], np.float32)
    bo = np.asarray(inputs["bo"], np.float32)
    cvec = (bv @ Wo.T + bo).astype(np.float32)   # exact: attn rows sum to 1
    out = np.empty((B, TQ, D), np.float32)
    for b in range(B):
        acc = np.zeros((D, TQ), np.float32)
        for g in range(NH_CORE):
            acc += np.asarray(results[b * NH_CORE + g]["oT"], np.float32)
        out[b] = acc.T + cvec
    return out


_NC_CACHE = None


def _get_nc():
    global _NC_CACHE
    if _NC_CACHE is None:
        _NC_CACHE = _build_nc()
    return _NC_CACHE


_RUNNER = None


def _get_runner():
    """Reusable jitted PJRT executable (same lowering run_bass_kernel_spmd
    uses under axon) so repeated kernel() calls skip recompilation."""
    global _RUNNER
    if _RUNNER is not None:
        return _RUNNER
    import jax
    from jax.sharding import Mesh, PartitionSpec
    try:
        from jax.experimental.shard_map import shard_map
    except ImportError:
        from jax import shard_map
    from concourse import bass2jax

    nc = _get_nc()
    bass2jax.install_neuronx_cc_hook()
    partition_name = (nc.partition_id_tensor.name
                      if nc.partition_id_tensor else None)
    in_names, out_names, out_avals, zero_outs = [], [], [], []
    for alloc in nc.m.functions[0].allocations:
        if not isinstance(alloc, mybir.MemoryLocationSet):
            continue
        name = alloc.memorylocations[0].name
        if alloc.kind == "ExternalInput":
            if name != partition_name:
                in_names.append(name)
        elif alloc.kind == "ExternalOutput":
            out_names.append(name)
            shape = tuple(alloc.tensor_shape)
            dtype = mybir.dt.np(alloc.dtype)
            out_avals.append(jax.core.ShapedArray(shape, dtype))
            zero_outs.append(np.zeros(shape, dtype))
    n_params = len(in_names)
    all_in = list(in_names) + list(out_names)
    if partition_name is not None:
        all_in.append(partition_name)

    def _b(*args):
        operands = list(args)
        if partition_name is not None:
            operands.append(bass2jax.partition_id_tensor())
        return tuple(bass2jax._bass_exec_p.bind(
            *operands, out_avals=tuple(out_avals), in_names=tuple(all_in),
            out_names=tuple(out_names), lowering_input_output_aliases=(),
            sim_require_finite=True, sim_require_nnan=True, nc=nc))

    devices = jax.devices()[:N_CORES]
    mesh = Mesh(np.asarray(devices), ("core",))
    nio = n_params + len(out_avals)
    fn = jax.jit(shard_map(_b, mesh=mesh,
                           in_specs=(PartitionSpec("core"),) * nio,
                           out_specs=(PartitionSpec("core"),) * len(out_avals),
                           check_rep=False), keep_unused=True)

    def run(in_maps):
        per_core = [[np.asarray(m[n]) for n in in_names] for m in in_maps]
        concat_in = [np.concatenate([per_core[c][i] for c in range(N_CORES)],
                                    axis=0) for i in range(n_params)]
        concat_zeros = [np.zeros((N_CORES * z.shape[0], *z.shape[1:]), z.dtype)
                        for z in zero_outs]
        outs = fn(*concat_in, *concat_zeros)
        return [
            {name: np.asarray(outs[i]).reshape(N_CORES, *out_avals[i].shape)[c]
             for i, name in enumerate(out_names)}
            for c in range(N_CORES)
        ]

    _RUNNER = run
    return run


_CALLED = False


def kernel(**inputs) -> np.ndarray:
    """Full-input entry point: shards across 8 NeuronCores, runs the Bass
    kernel, gathers and unshards. First call uses run_bass_kernel_spmd
    (compile + run); later calls reuse the cached executable."""
    global _CALLED
    in_maps = _make_in_maps(inputs)
    if not _CALLED:
        _CALLED = True
        nc = _get_nc()
        res = run_bass_kernel_spmd(nc, in_maps, list(range(N_CORES)))
        results = res.results
    else:
        results = _get_runner()(in_maps)
    return _assemble(results, inputs)


# revision 24
# speedup vs baseline: 1.1879x; 1.1879x over previous
"""Trainium2 Bass kernel for a cross-attention block with 3D-coordinate RoPE.

Module: q/k/v projections of x [B,Tq,D] against memory [B,Tk,D], 3D-coord
rotary embedding on q/k, softmax(q k^T / sqrt(Hd)) v, output projection.
B=2, Tq=1024, Tk=2048, D=1536, 16 heads x 96.

Sharding: 8 cores = (2 batches) x (4 head-groups of 4 heads). Each core
computes its heads end-to-end plus a partial output projection; the host
sums the 4 partials per batch. Biases bv/bo are folded in on the host
(attention rows sum to one), bq/bk are added on-device during PSUM
eviction.

Layout: feature-major ("transposed") on device. Scores are computed
transposed (S^T = k q^T) so the PV matmul needs no on-chip transposes;
softmax denominators come from a ones-column appended to v; the
per-query normalization is broadcast across partitions with a K=1
ones-vector matmul + full-lane approximate reciprocal.

Schedule notes (v2):
 - PE warm-up matmuls run during the initial DMA flight so the HAM clock
   gate reaches 8/8 before the Q projection starts.
 - Input DMAs are spread across three issue queues (sync: wq/x, gpsimd:
   memory chunks, scalar: weights/tables) so the Q-phase dependency chain
   is not head-of-line blocked behind the K/V prefetch.
 - The K projection is chunk-outer (contraction-major) over column
   halves, consuming memory chunks in DMA arrival order.
 - RoPE uses an SBUF->SBUF DMA partition swap instead of a PE
   permutation matmul; the multiply/add chain runs on DVE in fp16.
 - Output is written fp16 (host accumulates partials in fp32).
"""

import os
import sys

sys.path.insert(0, "/opt/trn_rl_repo")

import numpy as np
import ml_dtypes
from contextlib import ExitStack

import concourse.bass as bass
import concourse.tile as tile
from concourse import bacc, mybir
from concourse.bass_utils import run_bass_kernel_spmd

# ---------------------------------------------------------------- constants
B = 2
TQ = 1024
TK = 2048
D = 1536
NH = 16
HD = 96
ROPE_HALF = HD // 2           # 48
FREQ_PER_AXIS = ROPE_HALF // 3  # 16
ROPE_BASE = 10000.0
NH_CORE = 4                   # heads per core
HG = NH_CORE * HD             # 384 features per core
KC = D // 128                 # 12 contraction chunks
MTILES = D // 128             # 12 output-row tiles of the o-projection
SCALE = 1.0 / float(np.sqrt(HD))
N_CORES = 8
VW = HD + 1                   # 97: head-dim + ones column

F32 = mybir.dt.float32

_MM_DT_NAME = os.environ.get("KMM_DTYPE", "f16")
_DT = {"bf16": mybir.dt.bfloat16, "f16": mybir.dt.float16}
_NP = {"bf16": ml_dtypes.bfloat16, "f16": np.float16}
QK_DT = PV_DT = _DT[_MM_DT_NAME]
QK_NP = PV_NP = _NP[_MM_DT_NAME]


# ---------------------------------------------------------------- bass build
def _build_nc():
    nc = bacc.Bacc(trn_type="TRN2", target_bir_lowering=False, debug=False)

    io = {}
    def dram_in(name, shape, dt):
        io[name] = nc.dram_tensor(name, list(shape), dt, kind="ExternalInput").ap()
    dram_in("xT", [D, TQ], QK_DT)
    dram_in("memT", [D, TK], QK_DT)
    dram_in("wqT", [D, HG], QK_DT)  # columns of Wq^T for this head group
    dram_in("wkT", [D, HG], QK_DT)
    dram_in("wvT", [D, HG], PV_DT)
    dram_in("woT", [HG, D], PV_DT)  # rows of Wo^T for this head group
    dram_in("bq4", [HD, NH_CORE], F32)
    dram_in("bk4", [HD, NH_CORE], F32)
    dram_in("cqE", [HD, TQ], QK_DT)  # cos table, feature-major, q side
    dram_in("sqE", [HD, TQ], QK_DT)  # sign-folded sin table, q side
    dram_in("ckE", [HD, TK], QK_DT)
    dram_in("skE", [HD, TK], QK_DT)
    oT = nc.dram_tensor("oT", [D, TQ], PV_DT, kind="ExternalOutput").ap()

    with tile.TileContext(nc) as tc, ExitStack() as ctx:
        _body(ctx, tc, io, oT)
    nc.compile()
    return nc


def _body(ctx, tc, io, oT):
    nc = tc.nc
    P = 128
    NKC = TK // P
    Exp = mybir.ActivationFunctionType.Exp
    Ident = mybir.ActivationFunctionType.Identity

    const = ctx.enter_context(tc.tile_pool(name="const", bufs=1))
    resident = ctx.enter_context(tc.tile_pool(name="resident", bufs=1))

    ones1_t = const.tile([1, P], PV_DT, name="ones1_t")
    bq_t = const.tile([HD, NH_CORE], F32, name="bq_t")
    bk_t = const.tile([HD, NH_CORE], F32, name="bk_t")
    cq_t = const.tile([HD, TQ], QK_DT, name="cq_t")
    sq_t = const.tile([HD, TQ], QK_DT, name="sq_t")
    ck_t = const.tile([HD, TK], QK_DT, name="ck_t")
    sk_t = const.tile([HD, TK], QK_DT, name="sk_t")
    wk_all = const.tile([P, KC, HG], QK_DT, name="wk_all")
    wv_all = const.tile([P, KC, HG], PV_DT, name="wv_all")
    qT = [resident.tile([HD, TQ], QK_DT, name=f"qT{h}", tag=f"qT{h}")
          for h in range(NH_CORE)]
    kT = [resident.tile([HD, TK], QK_DT, name=f"kT{h}", tag=f"kT{h}")
          for h in range(NH_CORE)]
    vst = [resident.tile([P, NH_CORE * VW], PV_DT, name=f"vst{m}", tag=f"vst{m}")
           for m in range(NKC)]
    tmp_pool = ctx.enter_context(tc.tile_pool(name="tmp_pool", bufs=4))
    p_pool = ctx.enter_context(tc.tile_pool(name="p_pool", bufs=14))
    # full memory^T resident: loaded once, serves K and V
    mem_stack = ExitStack()
    mem_pool = mem_stack.enter_context(tc.tile_pool(name="mem_pool", bufs=1))
    memR = [mem_pool.tile([P, TK], QK_DT, name=f"memR{c}", tag=f"memR{c}")
            for c in range(KC)]
    # swap-source tiles for RoPE, freed after the rope phase (opened after
    # mem_pool: pools close LIFO and sw closes first)
    sw_stack = ExitStack()
    sw_pool = sw_stack.enter_context(tc.tile_pool(name="sw_pool", bufs=1))
    qsw = [sw_pool.tile([HD, TQ], QK_DT, name=f"qsw{h}", tag=f"qsw{h}")
           for h in range(NH_CORE)]
    # ksw is half-width: reused for the second K column-half (WAR dep on
    # the first half's rope multiply keeps it correct)
    ksw = [sw_pool.tile([HD, TK // 2], QK_DT, name=f"ksw{h}", tag=f"ksw{h}")
           for h in range(NH_CORE)]

    # ---- PE warm-up: matmuls on memset tiles while input DMAs fly ---------
    with ExitStack() as wctx:
        warm_sb = wctx.enter_context(tc.tile_pool(name="warm_sb", bufs=1))
        warm_ps = wctx.enter_context(
            tc.tile_pool(name="warm_ps", bufs=1, space="PSUM"))
        wz = warm_sb.tile([P, 512], QK_DT, name="wz")
        nc.gpsimd.memset(wz[:], 0.0)
        wps = warm_ps.tile([P, 512], F32, name="wps", tag="wps")
        for _ in range(14):
            nc.tensor.matmul(wps[:], wz[:, 0:P], wz[:], start=True, stop=True)

    # ones for the denominator broadcast + v ones-columns (no DMA needed)
    nc.gpsimd.memset(ones1_t[:], 1.0)
    for m in range(NKC):
        ones_cols = vst[m].rearrange("p (h c) -> p h c", c=VW)[:, :, HD:HD + 1]
        nc.gpsimd.memset(ones_cols, 1.0)

    # ---- input DMA issue ----------------------------------------------
    # DMA service across queues is roughly issue-ordered, so the bulk
    # input stream goes on ONE queue (sync) in exact consumption order;
    # the xq pool's buffer rotation paces the later issues automatically.
    # Tiny biases/tables ride the scalar queue up front.
    nc.scalar.dma_start(out=bq_t[:], in_=io["bq4"][:])
    nc.scalar.dma_start(out=cq_t[:], in_=io["cqE"][:])
    nc.scalar.dma_start(out=sq_t[:], in_=io["sqE"][:])
    nc.scalar.dma_start(out=bk_t[:], in_=io["bk4"][:])
    nc.scalar.dma_start(out=ck_t[:], in_=io["ckE"][:])
    nc.scalar.dma_start(out=sk_t[:], in_=io["skE"][:])

    def rope(dst, swp, cE, sE, lo, width, swp_lo=None):
        """RoPE on dst[:, lo:lo+width] via DMA partition swap + DVE fp16.

        swp[j] = dst[(j+48) % 96]; dst <- dst*cE + swp*sE (sE sign-folded).
        """
        if swp_lo is None:
            swp_lo = lo
        sl = slice(lo, lo + width)
        ssl = slice(swp_lo, swp_lo + width)
        nc.sync.dma_start(out=swp[0:ROPE_HALF, ssl],
                          in_=dst[ROPE_HALF:HD, sl])
        nc.sync.dma_start(out=swp[ROPE_HALF:HD, ssl],
                          in_=dst[0:ROPE_HALF, sl])
        t1 = tmp_pool.tile([HD, width], QK_DT, name="t1", tag="tmp")
        t2 = tmp_pool.tile([HD, width], QK_DT, name="t2", tag="tmp")
        nc.vector.tensor_mul(t1[:], dst[:, sl], cE[:, sl])
        nc.vector.tensor_mul(t2[:], swp[:, ssl], sE[:, sl])
        nc.vector.tensor_add(dst[:, sl], t1[:], t2[:])

    # ---- phase Q: q^T = Wq_h @ x^T (c-outer, 8 psum banks) ---------------
    with ExitStack() as qctx:
        psq_pool = qctx.enter_context(
            tc.tile_pool(name="psq", bufs=NH_CORE, space="PSUM"))
        xq_pool = qctx.enter_context(tc.tile_pool(name="xq", bufs=3))
        wq_pool = qctx.enter_context(tc.tile_pool(name="wq", bufs=1))
        wq_all = wq_pool.tile([P, KC, HG], QK_DT, name="wq_all")
        psq = [psq_pool.tile([HD, TQ], F32, name=f"psq{h}", tag="psq")
               for h in range(NH_CORE)]
        # bulk input stream in consumption order: Q pairs, then wv (for
        # Vh0), memory chunks, wk thirds (for K), interleaved.  The xq
        # pool (3 bufs) blocks the c>=3 issues until Q consumes, which
        # paces everything behind them.
        xcs = []
        def q_pair(c):
            nc.sync.dma_start(out=wq_all[:, c, :],
                              in_=io["wqT"][c * P:(c + 1) * P, :])
            xc = xq_pool.tile([P, TQ], QK_DT, name="xc", tag="xc")
            nc.sync.dma_start(out=xc[:], in_=io["xT"][c * P:(c + 1) * P, :])
            xcs.append(xc)
        def mem_issue(cm):
            nc.sync.dma_start(out=memR[cm][:],
                              in_=io["memT"][cm * P:(cm + 1) * P, :])
        def wk_issue(i):
            nc.sync.dma_start(
                out=wk_all[:, 4 * i:4 * (i + 1), :],
                in_=io["wkT"][512 * i:512 * (i + 1), :]
                .rearrange("(c p) h -> p c h", p=P))
        for c in range(3):
            q_pair(c)
        nc.sync.dma_start(
            out=wv_all[:],
            in_=io["wvT"][:].rearrange("(c p) h -> p c h", p=P))
        for c in range(3, KC):
            q_pair(c)
            if c >= 4:
                mem_issue(c - 4)
        for i, cm in ((0, 8), (1, 9), (2, 10)):
            wk_issue(i)
            mem_issue(cm)
        mem_issue(11)
        for c in range(KC):
            for h in range(NH_CORE):
                lhs = wq_all[:, c, h * HD:(h + 1) * HD]
                for n in range(2):
                    nc.tensor.matmul(
                        psq[h][:, n * 512:(n + 1) * 512],
                        lhs, xcs[c][:, n * 512:(n + 1) * 512],
                        start=(c == 0), stop=(c == KC - 1))
        for h in range(NH_CORE):
            if h % 2 == 0:
                nc.vector.tensor_scalar_add(qT[h][:], psq[h][:],
                                            bq_t[:, h:h + 1])
            else:
                nc.scalar.activation(qT[h][:], psq[h][:], Ident,
                                     bias=bq_t[:, h:h + 1])

    # rope-q: swap DMAs on sync (free after Q loads), DVE muls overlap V/K
    for h in range(NH_CORE):
        rope(qT[h], qsw[h], cq_t, sq_t, 0, TQ)

    # ---- phase V first half: v rows 0..1023, chunk-outer ------------------
    # Runs between Q and K so the PE chews on V while the memory prefetch
    # completes; consumes mem chunks in DMA arrival order. Two passes of 4
    # row-blocks so the evictions of pass A overlap the matmuls of pass B.
    with ExitStack() as vctx0:
        psv8_pool = vctx0.enter_context(
            tc.tile_pool(name="psv8", bufs=8, space="PSUM"))
        for pas in range(2):
            psv = [psv8_pool.tile([P, HG], F32, name=f"psv8_{pas}_{i}",
                                  tag="psv8") for i in range(4)]
            for c in range(KC):
                for i in range(4):
                    mg = pas * 4 + i
                    nc.tensor.matmul(
                        psv[i][:], memR[c][:, mg * P:(mg + 1) * P],
                        wv_all[:, c, :],
                        start=(c == 0), stop=(c == KC - 1))
            for i in range(4):
                mg = pas * 4 + i
                dst = vst[mg].rearrange("p (h c) -> p h c", c=VW)[:, :, 0:HD]
                src = psv[i].rearrange("p (h c) -> p h c", c=HD)
                nc.vector.tensor_copy(dst, src)

    # ---- phase K: k^T = Wk_h @ mem^T (chunk-outer over column halves) ----
    with ExitStack() as kctx:
        psk_pool = kctx.enter_context(
            tc.tile_pool(name="psk", bufs=8, space="PSUM"))
        for half in range(2):
            base = half * 1024
            psk = [psk_pool.tile([HD, 512], F32, name=f"psk{half}_{i}",
                                 tag="psk") for i in range(8)]
            for c in range(KC):
                for h in range(NH_CORE):
                    lhs = wk_all[:, c, h * HD:(h + 1) * HD]
                    for qq in range(2):
                        col = base + qq * 512
                        nc.tensor.matmul(
                            psk[qq * NH_CORE + h][:], lhs,
                            memR[c][:, col:col + 512],
                            start=(c == 0), stop=(c == KC - 1))
            for qq in range(2):
                col = base + qq * 512
                for h in range(NH_CORE):
                    if (h + qq) % 2 == 0:
                        nc.vector.tensor_scalar_add(
                            kT[h][:, col:col + 512],
                            psk[qq * NH_CORE + h][:], bk_t[:, h:h + 1])
                    else:
                        nc.scalar.activation(
                            kT[h][:, col:col + 512],
                            psk[qq * NH_CORE + h][:], Ident,
                            bias=bk_t[:, h:h + 1])
            for h in range(NH_CORE):
                rope(kT[h], ksw[h], ck_t, sk_t, base, 1024, swp_lo=0)
    sw_stack.close()

    # attention score/exp pool opens before V so S/exp of head 0 can
    # overlap the V matmuls (ACT is otherwise idle during V)
    s_ps = ctx.enter_context(tc.tile_pool(name="s_ps", bufs=3, space="PSUM"))

    PTS = {}

    def emit_schunk(h, kc, _unused=None):
        st = s_ps.tile([P, TQ], F32, name="st", tag="s")
        lhs = kT[h][:, kc * P:(kc + 1) * P]
        nc.tensor.matmul(st[:, 0:512], lhs, qT[h][:, 0:512])
        nc.tensor.matmul(st[:, 512:1024], lhs, qT[h][:, 512:1024])
        pt = p_pool.tile([P, TQ], PV_DT, name="pt", tag="pt")
        nc.scalar.activation(pt[:], st[:], Exp, scale=SCALE)
        PTS[(h, kc)] = pt

    PRE = 14

    # ---- phase V second half: v rows 1024..2047 (m-outer, c-inner) -------
    # All of memory is resident by now; head-0 scores interleave so the
    # exp pipeline on ACT starts well before the flat attention loop.
    with ExitStack() as vctx:
        psv_pool = vctx.enter_context(
            tc.tile_pool(name="psv", bufs=2, space="PSUM"))
        for q8 in range(4, 8):
            col = q8 * 256
            psv = [psv_pool.tile([P, HG], F32, name=f"psv{q8}_{i}", tag="psv")
                   for i in range(2)]
            for c in range(KC):
                for ml in range(2):
                    nc.tensor.matmul(
                        psv[ml][:], memR[c][:, col + ml * P:col + (ml + 1) * P],
                        wv_all[:, c, :],
                        start=(c == 0), stop=(c == KC - 1))
            for ml in range(2):
                mg = q8 * 2 + ml
                dst = vst[mg].rearrange("p (h c) -> p h c", c=VW)[:, :, 0:HD]
                src = psv[ml].rearrange("p (h c) -> p h c", c=HD)
                nc.vector.tensor_copy(dst, src)
            lo = [0, 4, 8, 11][q8 - 4]
            hi = [4, 8, 11, PRE][q8 - 4]
            for k in range(lo, hi):
                emit_schunk(0, k)
            if q8 == 4:
                wo_t = []
                for i in range(3):
                    w = const.tile([P, D], PV_DT, name=f"wo_t{i}",
                                   tag=f"wo_t{i}")
                    nc.scalar.dma_start(out=w[:],
                                        in_=io["woT"][i * P:(i + 1) * P, :])
                    wo_t.append(w)
    mem_stack.close()

    # ---- attention: one flat pipeline across all (head, chunk) pairs -----
    # PV lags S/exp by 3 chunks and flows straight across head boundaries,
    # so the ACT engine never drains between heads.
    pv_ps = ctx.enter_context(tc.tile_pool(name="pv_ps", bufs=2, space="PSUM"))
    aout_pool = ctx.enter_context(tc.tile_pool(name="aout_pool", bufs=2))
    aN_pool = ctx.enter_context(tc.tile_pool(name="aN_pool", bufs=1))
    ot_pool = ctx.enter_context(tc.tile_pool(name="ot_pool", bufs=3))

    # aoutN stacked as 3 tiles of 128 partitions (heads packed) so the
    # o-projection contracts in 3 chunks of 128 instead of 4 of 96
    aN = [aN_pool.tile([P, TQ], PV_DT, name=f"aN{i}", tag=f"aN{i}")
          for i in range(3)]
    # per-head write segments: (tile, tile_row0, head_row0, nrows)
    _SEG = {0: [(0, 0, 0, 96)],
            1: [(0, 96, 0, 32), (1, 0, 32, 32), (1, 32, 64, 32)],
            2: [(1, 64, 0, 64), (2, 0, 64, 32)],
            3: [(2, 32, 0, 32), (2, 64, 32, 32), (2, 96, 64, 32)]}

    pvs = {}

    def finish_head(h):
        pv0, pv1 = pvs.pop(h)
        aout = aout_pool.tile([VW, TQ], PV_DT, name="aout", tag="aout")
        # denominator row straight from PSUM so the broadcast matmul does
        # not serialize behind the aout eviction
        den1 = tmp_pool.tile([1, TQ], PV_DT, name="den1", tag="den1")
        nc.vector.tensor_copy(den1[:, 0:512], pv0[HD:HD + 1, :])
        nc.vector.tensor_copy(den1[:, 512:1024], pv1[HD:HD + 1, :])
        nc.vector.tensor_copy(aout[:, 0:512], pv0[:])
        nc.vector.tensor_copy(aout[:, 512:1024], pv1[:])
        denB = s_ps.tile([P, TQ], F32, name="denB", tag="s")
        for n in range(2):
            nc.tensor.matmul(denB[:, n * 512:(n + 1) * 512], ones1_t[:],
                             den1[:, n * 512:(n + 1) * 512])
        recB = tmp_pool.tile([HD, TQ], F32, name="recB", tag="tmp")
        nc.vector.reciprocal_approx_fast(out=recB[:], in_=denB[0:HD, :])
        for (ti, tr, hr, nr) in _SEG[h]:
            nc.vector.tensor_mul(aN[ti][tr:tr + nr, :],
                                 aout[hr:hr + nr, :], recB[hr:hr + nr, :])

    LAG = 3
    G = NH_CORE * NKC
    for g in range(G + LAG):
        if g < G:
            h, kc = divmod(g, NKC)
            if not (h == 0 and kc < PRE):
                emit_schunk(h, kc)
        if g >= LAG:
            h2, kc2 = divmod(g - LAG, NKC)
            if kc2 == 0:
                pvs[h2] = (
                    pv_ps.tile([VW, 512], F32, name=f"pv{h2}0", tag="pv"),
                    pv_ps.tile([VW, 512], F32, name=f"pv{h2}1", tag="pv"))
            pv0, pv1 = pvs[h2]
            pt = PTS[(h2, kc2)]
            vl = vst[kc2][:, h2 * VW:(h2 + 1) * VW]
            first, last = (kc2 == 0), (kc2 == NKC - 1)
            nc.tensor.matmul(pv0[:], vl, pt[:, 0:512], start=first, stop=last)
            nc.tensor.matmul(pv1[:], vl, pt[:, 512:1024],
                             start=first, stop=last)
            PTS.pop((h2, kc2))
            if last:
                finish_head(h2)

    # ---- output projection (h-inner accumulation, 2 matmuls per weight) --
    for m in range(MTILES):
        po0 = s_ps.tile([P, 512], F32, name="po0", tag="s")
        po1 = s_ps.tile([P, 512], F32, name="po1", tag="s")
        for i in range(3):
            lhs = wo_t[i][:, m * P:(m + 1) * P]
            nc.tensor.matmul(po0[:], lhs, aN[i][:, 0:512],
                             start=(i == 0), stop=(i == 2))
            nc.tensor.matmul(po1[:], lhs, aN[i][:, 512:1024],
                             start=(i == 0), stop=(i == 2))
        ot = ot_pool.tile([P, TQ], PV_DT, name="ot", tag="ot")
        nc.vector.tensor_copy(ot[:, 0:512], po0[:])
        nc.scalar.copy(ot[:, 512:1024], po1[:])
        eng = nc.sync if m % 2 == 0 else nc.gpsimd
        eng.dma_start(out=oT[m * P:(m + 1) * P, :], in_=ot[:])


# ---------------------------------------------------------------- host side
def _rope_tables(coords, T):
    """Feature-major cos/sin tables [HD, T] with the sign fold.

    Row j < 48 of the rotated output is q[j]*cos_j - q[j+48]*sin_j and row
    j >= 48 is q[j]*cos_{j-48} + q[j-48]*sin_{j-48}; the device computes
    rot = q * cE + swap(q) * sE with swap(q)[j] = q[(j+48) % 96].
    """
    coords = np.asarray(coords, np.float32)
    inv_freq = (1.0 / (ROPE_BASE ** (np.arange(FREQ_PER_AXIS, dtype=np.float32)
                                     / FREQ_PER_AXIS))).astype(np.float32)
    ang = coords[:, :, None] * inv_freq[None, None, :]   # [T, 3, 16]
    ang = ang.reshape(T, ROPE_HALF)                      # [T, 48]
    sin = np.sin(ang).astype(np.float32).T               # [48, T]
    cos = np.cos(ang).astype(np.float32).T
    cE = np.concatenate([cos, cos], axis=0)              # [96, T]
    sE = np.concatenate([-sin, sin], axis=0)
    return (np.ascontiguousarray(cE).astype(QK_NP),
            np.ascontiguousarray(sE).astype(QK_NP))


def _make_in_maps(inputs):
    x = np.asarray(inputs["x"], np.float32)
    memory = np.asarray(inputs["memory"], np.float32)
    qc = np.asarray(inputs["query_coords"], np.float32)
    mc = np.asarray(inputs["memory_coords"], np.float32)
    Wq = np.asarray(inputs["Wq"], np.float32)
    Wk = np.asarray(inputs["Wk"], np.float32)
    Wv = np.asarray(inputs["Wv"], np.float32)
    Wo = np.asarray(inputs["Wo"], np.float32)
    bq = np.asarray(inputs["bq"], np.float32)
    bk = np.asarray(inputs["bk"], np.float32)

    WqT = np.ascontiguousarray(Wq.T).astype(QK_NP)   # [in, out]
    WkT = np.ascontiguousarray(Wk.T).astype(QK_NP)
    WvT = np.ascontiguousarray(Wv.T).astype(PV_NP)
    WoT = np.ascontiguousarray(Wo.T).astype(PV_NP)

    per_batch = []
    for b in range(B):
        cqE, sqE = _rope_tables(qc[b], TQ)
        ckE, skE = _rope_tables(mc[b], TK)
        entry = {
            "xT": np.ascontiguousarray(x[b].T).astype(QK_NP),
            "memT": np.ascontiguousarray(memory[b].T).astype(QK_NP),
            "cqE": cqE, "sqE": sqE, "ckE": ckE, "skE": skE,
        }
        per_batch.append(entry)

    in_maps = []
    for core in range(N_CORES):
        b, g = divmod(core, NH_CORE)
        sl = slice(g * HG, (g + 1) * HG)
        m = dict(per_batch[b])
        m["wqT"] = np.ascontiguousarray(WqT[:, sl])
        m["wkT"] = np.ascontiguousarray(WkT[:, sl])
        m["wvT"] = np.ascontiguousarray(WvT[:, sl])
        m["woT"] = np.ascontiguousarray(WoT[sl, :])
        m["bq4"] = np.ascontiguousarray(bq[sl].reshape(NH_CORE, HD).T)
        m["bk4"] = np.ascontiguousarray(bk[sl].reshape(NH_CORE, HD).T)
        in_maps.append(m)
    return in_maps


def _assemble(results, inputs):
    Wo = np.asarray(inputs["Wo"], np.float32)
    bv = np.asarray(inputs["bv"], np.float32)
    bo = np.asarray(inputs["bo"], np.float32)
    cvec = (bv @ Wo.T + bo).astype(np.float32)   # exact: attn rows sum to 1
    out = np.empty((B, TQ, D), np.float32)
    for b in range(B):
        acc = np.zeros((D, TQ), np.float32)
        for g in range(NH_CORE):
            acc += np.asarray(results[b * NH_CORE + g]["oT"], np.float32)
        out[b] = acc.T + cvec
    return out


_NC_CACHE = None


def _get_nc():
    global _NC_CACHE
    if _NC_CACHE is None:
        _NC_CACHE = _build_nc()
    return _NC_CACHE


_RUNNER = None


def _get_runner():
    """Reusable jitted PJRT executable (same lowering run_bass_kernel_spmd
    uses under axon) so repeated kernel() calls skip recompilation."""
    global _RUNNER
    if _RUNNER is not None:
        return _RUNNER
    import jax
    from jax.sharding import Mesh, PartitionSpec
    try:
        from jax.experimental.shard_map import shard_map
    except ImportError:
        from jax import shard_map
    from concourse import bass2jax

    nc = _get_nc()
    bass2jax.install_neuronx_cc_hook()
    partition_name = (nc.partition_id_tensor.name
                      if nc.partition_id_tensor else None)
    in_names, out_names, out_avals, zero_outs = [], [], [], []
    for alloc in nc.m.functions[0].allocations:
        if not isinstance(alloc, mybir.MemoryLocationSet):
            continue
        name = alloc.memorylocations[0].name
        if alloc.kind == "ExternalInput":
            if name != partition_name:
                in_names.append(name)
        elif alloc.kind == "ExternalOutput":
            out_names.append(name)
            shape = tuple(alloc.tensor_shape)
            dtype = mybir.dt.np(alloc.dtype)
            out_avals.append(jax.core.ShapedArray(shape, dtype))
            zero_outs.append(np.zeros(shape, dtype))
    n_params = len(in_names)
    all_in = list(in_names) + list(out_names)
    if partition_name is not None:
        all_in.append(partition_name)

    def _b(*args):
        operands = list(args)
        if partition_name is not None:
            operands.append(bass2jax.partition_id_tensor())
        return tuple(bass2jax._bass_exec_p.bind(
            *operands, out_avals=tuple(out_avals), in_names=tuple(all_in),
            out_names=tuple(out_names), lowering_input_output_aliases=(),
            sim_require_finite=True, sim_require_nnan=True, nc=nc))

    devices = jax.devices()[:N_CORES]
    mesh = Mesh(np.asarray(devices), ("core",))
    nio = n_params + len(out_avals)
    fn = jax.jit(shard_map(_b, mesh=mesh,
                           in_specs=(PartitionSpec("core"),) * nio,
                           out_specs=(PartitionSpec("core"),) * len(out_avals),
                           check_rep=False), keep_unused=True)

    def run(in_maps):
        per_core = [[np.asarray(m[n]) for n in in_names] for m in in_maps]
        concat_in = [np.concatenate([per_core[c][i] for c in range(N_CORES)],
                                    axis=0) for i in range(n_params)]
        concat_zeros = [np.zeros((N_CORES * z.shape[0], *z.shape[1:]), z.dtype)
                        for z in zero_outs]
        outs = fn(*concat_in, *concat_zeros)
        return [
            {name: np.asarray(outs[i]).reshape(N_CORES, *out_avals[i].shape)[c]
             for i, name in enumerate(out_names)}
            for c in range(N_CORES)
        ]

    _RUNNER = run
    return run


_CALLED = False


def kernel(**inputs) -> np.ndarray:
    """Full-input entry point: shards across 8 NeuronCores, runs the Bass
    kernel, gathers and unshards. First call uses run_bass_kernel_spmd
    (compile + run); later calls reuse the cached executable."""
    global _CALLED
    in_maps = _make_in_maps(inputs)
    if not _CALLED:
        _CALLED = True
        nc = _get_nc()
        res = run_bass_kernel_spmd(nc, in_maps, list(range(N_CORES)))
        results = res.results
    else:
        results = _get_runner()(in_maps)
    return _assemble(results, inputs)


# revision 25
# speedup vs baseline: 1.1892x; 1.0010x over previous
"""Trainium2 Bass kernel for a cross-attention block with 3D-coordinate RoPE.

Module: q/k/v projections of x [B,Tq,D] against memory [B,Tk,D], 3D-coord
rotary embedding on q/k, softmax(q k^T / sqrt(Hd)) v, output projection.
B=2, Tq=1024, Tk=2048, D=1536, 16 heads x 96.

Sharding: 8 cores = (2 batches) x (4 head-groups of 4 heads). Each core
computes its heads end-to-end plus a partial output projection; the host
sums the 4 partials per batch. Biases bv/bo are folded in on the host
(attention rows sum to one), bq/bk are added on-device during PSUM
eviction.

Layout: feature-major ("transposed") on device. Scores are computed
transposed (S^T = k q^T) so the PV matmul needs no on-chip transposes;
softmax denominators come from a ones-column appended to v; the
per-query normalization is broadcast across partitions with a K=1
ones-vector matmul + full-lane approximate reciprocal.

Schedule notes (v2):
 - PE warm-up matmuls run during the initial DMA flight so the HAM clock
   gate reaches 8/8 before the Q projection starts.
 - Input DMAs are spread across three issue queues (sync: wq/x, gpsimd:
   memory chunks, scalar: weights/tables) so the Q-phase dependency chain
   is not head-of-line blocked behind the K/V prefetch.
 - The K projection is chunk-outer (contraction-major) over column
   halves, consuming memory chunks in DMA arrival order.
 - RoPE uses an SBUF->SBUF DMA partition swap instead of a PE
   permutation matmul; the multiply/add chain runs on DVE in fp16.
 - Output is written fp16 (host accumulates partials in fp32).
"""

import os
import sys

sys.path.insert(0, "/opt/trn_rl_repo")

import numpy as np
import ml_dtypes
from contextlib import ExitStack

import concourse.bass as bass
import concourse.tile as tile
from concourse import bacc, mybir
from concourse.bass_utils import run_bass_kernel_spmd

# ---------------------------------------------------------------- constants
B = 2
TQ = 1024
TK = 2048
D = 1536
NH = 16
HD = 96
ROPE_HALF = HD // 2           # 48
FREQ_PER_AXIS = ROPE_HALF // 3  # 16
ROPE_BASE = 10000.0
NH_CORE = 4                   # heads per core
HG = NH_CORE * HD             # 384 features per core
KC = D // 128                 # 12 contraction chunks
MTILES = D // 128             # 12 output-row tiles of the o-projection
SCALE = 1.0 / float(np.sqrt(HD))
N_CORES = 8
VW = HD + 1                   # 97: head-dim + ones column

F32 = mybir.dt.float32

_MM_DT_NAME = os.environ.get("KMM_DTYPE", "f16")
_DT = {"bf16": mybir.dt.bfloat16, "f16": mybir.dt.float16}
_NP = {"bf16": ml_dtypes.bfloat16, "f16": np.float16}
QK_DT = PV_DT = _DT[_MM_DT_NAME]
QK_NP = PV_NP = _NP[_MM_DT_NAME]


# ---------------------------------------------------------------- bass build
def _build_nc():
    nc = bacc.Bacc(trn_type="TRN2", target_bir_lowering=False, debug=False)

    io = {}
    def dram_in(name, shape, dt):
        io[name] = nc.dram_tensor(name, list(shape), dt, kind="ExternalInput").ap()
    dram_in("xT", [D, TQ], QK_DT)
    dram_in("memT", [D, TK], QK_DT)
    dram_in("wqT", [D, HG], QK_DT)  # columns of Wq^T for this head group
    dram_in("wkT", [D, HG], QK_DT)
    dram_in("wvT", [D, HG], PV_DT)
    dram_in("woT", [HG, D], PV_DT)  # rows of Wo^T for this head group
    dram_in("bq4", [HD, NH_CORE], F32)
    dram_in("bk4", [HD, NH_CORE], F32)
    dram_in("cqE", [HD, TQ], QK_DT)  # cos table, feature-major, q side
    dram_in("sqE", [HD, TQ], QK_DT)  # sign-folded sin table, q side
    dram_in("ckE", [HD, TK], QK_DT)
    dram_in("skE", [HD, TK], QK_DT)
    oT = nc.dram_tensor("oT", [D, TQ], PV_DT, kind="ExternalOutput").ap()

    with tile.TileContext(nc) as tc, ExitStack() as ctx:
        _body(ctx, tc, io, oT)
    nc.compile()
    return nc


def _body(ctx, tc, io, oT):
    nc = tc.nc
    P = 128
    NKC = TK // P
    Exp = mybir.ActivationFunctionType.Exp
    Ident = mybir.ActivationFunctionType.Identity

    const = ctx.enter_context(tc.tile_pool(name="const", bufs=1))
    resident = ctx.enter_context(tc.tile_pool(name="resident", bufs=1))

    ones1_t = const.tile([1, P], PV_DT, name="ones1_t")
    bq_t = const.tile([HD, NH_CORE], F32, name="bq_t")
    bk_t = const.tile([HD, NH_CORE], F32, name="bk_t")
    cq_t = const.tile([HD, TQ], QK_DT, name="cq_t")
    sq_t = const.tile([HD, TQ], QK_DT, name="sq_t")
    ck_t = const.tile([HD, TK], QK_DT, name="ck_t")
    sk_t = const.tile([HD, TK], QK_DT, name="sk_t")
    wk_all = const.tile([P, KC, HG], QK_DT, name="wk_all")
    wv_all = const.tile([P, KC, HG], PV_DT, name="wv_all")
    qT = [resident.tile([HD, TQ], QK_DT, name=f"qT{h}", tag=f"qT{h}")
          for h in range(NH_CORE)]
    kT = [resident.tile([HD, TK], QK_DT, name=f"kT{h}", tag=f"kT{h}")
          for h in range(NH_CORE)]
    vst = [resident.tile([P, NH_CORE * VW], PV_DT, name=f"vst{m}", tag=f"vst{m}")
           for m in range(NKC)]
    tmp_pool = ctx.enter_context(tc.tile_pool(name="tmp_pool", bufs=4))
    p_pool = ctx.enter_context(tc.tile_pool(name="p_pool", bufs=14))
    # full memory^T resident: loaded once, serves K and V
    mem_stack = ExitStack()
    mem_pool = mem_stack.enter_context(tc.tile_pool(name="mem_pool", bufs=1))
    memR = [mem_pool.tile([P, TK], QK_DT, name=f"memR{c}", tag=f"memR{c}")
            for c in range(KC)]
    # swap-source tiles for RoPE, freed after the rope phase (opened after
    # mem_pool: pools close LIFO and sw closes first)
    sw_stack = ExitStack()
    sw_pool = sw_stack.enter_context(tc.tile_pool(name="sw_pool", bufs=1))
    qsw = [sw_pool.tile([HD, TQ], QK_DT, name=f"qsw{h}", tag=f"qsw{h}")
           for h in range(NH_CORE)]
    # ksw is half-width: reused for the second K column-half (WAR dep on
    # the first half's rope multiply keeps it correct)
    ksw = [sw_pool.tile([HD, TK // 2], QK_DT, name=f"ksw{h}", tag=f"ksw{h}")
           for h in range(NH_CORE)]

    # ---- PE warm-up: matmuls on memset tiles while input DMAs fly --------
    # wz lives in the const pool: a scoped pool here would hand its SBUF
    # range to the xq pool, making xc0's DMA wait for the last warm matmul.
    wz = const.tile([P, 512], QK_DT, name="wz")
    with ExitStack() as wctx:
        warm_ps = wctx.enter_context(
            tc.tile_pool(name="warm_ps", bufs=1, space="PSUM"))
        nc.gpsimd.memset(wz[:], 0.0)
        wps = warm_ps.tile([P, 512], F32, name="wps", tag="wps")
        for _ in range(12):
            nc.tensor.matmul(wps[:], wz[:, 0:P], wz[:], start=True, stop=True)

    # ones for the denominator broadcast + v ones-columns (no DMA needed)
    nc.gpsimd.memset(ones1_t[:], 1.0)
    for m in range(NKC):
        ones_cols = vst[m].rearrange("p (h c) -> p h c", c=VW)[:, :, HD:HD + 1]
        nc.gpsimd.memset(ones_cols, 1.0)

    # ---- input DMA issue ----------------------------------------------
    # DMA service across queues is roughly issue-ordered, so the bulk
    # input stream goes on ONE queue (sync) in exact consumption order;
    # the xq pool's buffer rotation paces the later issues automatically.
    # Tiny biases/tables ride the scalar queue up front.
    nc.scalar.dma_start(out=bq_t[:], in_=io["bq4"][:])
    nc.scalar.dma_start(out=cq_t[:], in_=io["cqE"][:])
    nc.scalar.dma_start(out=sq_t[:], in_=io["sqE"][:])
    nc.scalar.dma_start(out=bk_t[:], in_=io["bk4"][:])
    nc.scalar.dma_start(out=ck_t[:], in_=io["ckE"][:])
    nc.scalar.dma_start(out=sk_t[:], in_=io["skE"][:])

    def rope(dst, swp, cE, sE, lo, width, swp_lo=None):
        """RoPE on dst[:, lo:lo+width] via DMA partition swap + DVE fp16.

        swp[j] = dst[(j+48) % 96]; dst <- dst*cE + swp*sE (sE sign-folded).
        """
        if swp_lo is None:
            swp_lo = lo
        sl = slice(lo, lo + width)
        ssl = slice(swp_lo, swp_lo + width)
        nc.sync.dma_start(out=swp[0:ROPE_HALF, ssl],
                          in_=dst[ROPE_HALF:HD, sl])
        nc.sync.dma_start(out=swp[ROPE_HALF:HD, ssl],
                          in_=dst[0:ROPE_HALF, sl])
        t1 = tmp_pool.tile([HD, width], QK_DT, name="t1", tag="tmp")
        t2 = tmp_pool.tile([HD, width], QK_DT, name="t2", tag="tmp")
        nc.vector.tensor_mul(t1[:], dst[:, sl], cE[:, sl])
        nc.vector.tensor_mul(t2[:], swp[:, ssl], sE[:, sl])
        nc.vector.tensor_add(dst[:, sl], t1[:], t2[:])

    # ---- phase Q: q^T = Wq_h @ x^T (c-outer, 8 psum banks) ---------------
    with ExitStack() as qctx:
        psq_pool = qctx.enter_context(
            tc.tile_pool(name="psq", bufs=NH_CORE, space="PSUM"))
        xq_pool = qctx.enter_context(tc.tile_pool(name="xq", bufs=3))
        wq_pool = qctx.enter_context(tc.tile_pool(name="wq", bufs=1))
        wq_all = wq_pool.tile([P, KC, HG], QK_DT, name="wq_all")
        psq = [psq_pool.tile([HD, TQ], F32, name=f"psq{h}", tag="psq")
               for h in range(NH_CORE)]
        # bulk input stream in consumption order: Q pairs, then wv (for
        # Vh0), memory chunks, wk thirds (for K), interleaved.  The xq
        # pool (3 bufs) blocks the c>=3 issues until Q consumes, which
        # paces everything behind them.
        xcs = []
        def q_pair(c):
            nc.sync.dma_start(out=wq_all[:, c, :],
                              in_=io["wqT"][c * P:(c + 1) * P, :])
            xc = xq_pool.tile([P, TQ], QK_DT, name="xc", tag="xc")
            nc.sync.dma_start(out=xc[:], in_=io["xT"][c * P:(c + 1) * P, :])
            xcs.append(xc)
        def mem_issue(cm):
            nc.sync.dma_start(out=memR[cm][:],
                              in_=io["memT"][cm * P:(cm + 1) * P, :])
        def wk_issue(i):
            nc.sync.dma_start(
                out=wk_all[:, 4 * i:4 * (i + 1), :],
                in_=io["wkT"][512 * i:512 * (i + 1), :]
                .rearrange("(c p) h -> p c h", p=P))
        for c in range(3):
            q_pair(c)
        nc.sync.dma_start(
            out=wv_all[:],
            in_=io["wvT"][:].rearrange("(c p) h -> p c h", p=P))
        for c in range(3, KC):
            q_pair(c)
            if c >= 4:
                mem_issue(c - 4)
        for i, cm in ((0, 8), (1, 9), (2, 10)):
            wk_issue(i)
            mem_issue(cm)
        mem_issue(11)
        for c in range(KC):
            for h in range(NH_CORE):
                lhs = wq_all[:, c, h * HD:(h + 1) * HD]
                for n in range(2):
                    nc.tensor.matmul(
                        psq[h][:, n * 512:(n + 1) * 512],
                        lhs, xcs[c][:, n * 512:(n + 1) * 512],
                        start=(c == 0), stop=(c == KC - 1))
        for h in range(NH_CORE):
            if h % 2 == 0:
                nc.vector.tensor_scalar_add(qT[h][:], psq[h][:],
                                            bq_t[:, h:h + 1])
            else:
                nc.scalar.activation(qT[h][:], psq[h][:], Ident,
                                     bias=bq_t[:, h:h + 1])

    # rope-q: swap DMAs on sync (free after Q loads), DVE muls overlap V/K
    for h in range(NH_CORE):
        rope(qT[h], qsw[h], cq_t, sq_t, 0, TQ)

    # ---- phase V first half: v rows 0..1023, chunk-outer ------------------
    # Runs between Q and K so the PE chews on V while the memory prefetch
    # completes; consumes mem chunks in DMA arrival order. Two passes of 4
    # row-blocks so the evictions of pass A overlap the matmuls of pass B.
    with ExitStack() as vctx0:
        psv8_pool = vctx0.enter_context(
            tc.tile_pool(name="psv8", bufs=8, space="PSUM"))
        for pas in range(2):
            psv = [psv8_pool.tile([P, HG], F32, name=f"psv8_{pas}_{i}",
                                  tag="psv8") for i in range(4)]
            for c in range(KC):
                for i in range(4):
                    mg = pas * 4 + i
                    nc.tensor.matmul(
                        psv[i][:], memR[c][:, mg * P:(mg + 1) * P],
                        wv_all[:, c, :],
                        start=(c == 0), stop=(c == KC - 1))
            for i in range(4):
                mg = pas * 4 + i
                dst = vst[mg].rearrange("p (h c) -> p h c", c=VW)[:, :, 0:HD]
                src = psv[i].rearrange("p (h c) -> p h c", c=HD)
                nc.vector.tensor_copy(dst, src)

    # ---- phase K: k^T = Wk_h @ mem^T (chunk-outer over column halves) ----
    with ExitStack() as kctx:
        psk_pool = kctx.enter_context(
            tc.tile_pool(name="psk", bufs=8, space="PSUM"))
        for half in range(2):
            base = half * 1024
            psk = [psk_pool.tile([HD, 512], F32, name=f"psk{half}_{i}",
                                 tag="psk") for i in range(8)]
            for c in range(KC):
                for h in range(NH_CORE):
                    lhs = wk_all[:, c, h * HD:(h + 1) * HD]
                    for qq in range(2):
                        col = base + qq * 512
                        nc.tensor.matmul(
                            psk[qq * NH_CORE + h][:], lhs,
                            memR[c][:, col:col + 512],
                            start=(c == 0), stop=(c == KC - 1))
            for qq in range(2):
                col = base + qq * 512
                for h in range(NH_CORE):
                    if (h + qq) % 2 == 0:
                        nc.vector.tensor_scalar_add(
                            kT[h][:, col:col + 512],
                            psk[qq * NH_CORE + h][:], bk_t[:, h:h + 1])
                    else:
                        nc.scalar.activation(
                            kT[h][:, col:col + 512],
                            psk[qq * NH_CORE + h][:], Ident,
                            bias=bk_t[:, h:h + 1])
            for h in range(NH_CORE):
                rope(kT[h], ksw[h], ck_t, sk_t, base, 1024, swp_lo=0)
    sw_stack.close()

    # attention score/exp pool opens before V so S/exp of head 0 can
    # overlap the V matmuls (ACT is otherwise idle during V)
    s_ps = ctx.enter_context(tc.tile_pool(name="s_ps", bufs=3, space="PSUM"))

    PTS = {}

    def emit_schunk(h, kc, _unused=None):
        st = s_ps.tile([P, TQ], F32, name="st", tag="s")
        lhs = kT[h][:, kc * P:(kc + 1) * P]
        nc.tensor.matmul(st[:, 0:512], lhs, qT[h][:, 0:512])
        nc.tensor.matmul(st[:, 512:1024], lhs, qT[h][:, 512:1024])
        pt = p_pool.tile([P, TQ], PV_DT, name="pt", tag="pt")
        nc.scalar.activation(pt[:], st[:], Exp, scale=SCALE)
        PTS[(h, kc)] = pt

    PRE = 14

    # ---- phase V second half: v rows 1024..2047 (m-outer, c-inner) -------
    # All of memory is resident by now; head-0 scores interleave so the
    # exp pipeline on ACT starts well before the flat attention loop.
    with ExitStack() as vctx:
        psv_pool = vctx.enter_context(
            tc.tile_pool(name="psv", bufs=2, space="PSUM"))
        for q8 in range(4, 8):
            col = q8 * 256
            psv = [psv_pool.tile([P, HG], F32, name=f"psv{q8}_{i}", tag="psv")
                   for i in range(2)]
            for c in range(KC):
                for ml in range(2):
                    nc.tensor.matmul(
                        psv[ml][:], memR[c][:, col + ml * P:col + (ml + 1) * P],
                        wv_all[:, c, :],
                        start=(c == 0), stop=(c == KC - 1))
            for ml in range(2):
                mg = q8 * 2 + ml
                dst = vst[mg].rearrange("p (h c) -> p h c", c=VW)[:, :, 0:HD]
                src = psv[ml].rearrange("p (h c) -> p h c", c=HD)
                nc.vector.tensor_copy(dst, src)
            lo = [0, 4, 8, 11][q8 - 4]
            hi = [4, 8, 11, PRE][q8 - 4]
            for k in range(lo, hi):
                emit_schunk(0, k)
            if q8 == 4:
                wo_t = []
                for i in range(3):
                    w = const.tile([P, D], PV_DT, name=f"wo_t{i}",
                                   tag=f"wo_t{i}")
                    nc.scalar.dma_start(out=w[:],
                                        in_=io["woT"][i * P:(i + 1) * P, :])
                    wo_t.append(w)
    mem_stack.close()

    # ---- attention: one flat pipeline across all (head, chunk) pairs -----
    # PV lags S/exp by 3 chunks and flows straight across head boundaries,
    # so the ACT engine never drains between heads.
    pv_ps = ctx.enter_context(tc.tile_pool(name="pv_ps", bufs=2, space="PSUM"))
    aout_pool = ctx.enter_context(tc.tile_pool(name="aout_pool", bufs=2))
    aN_pool = ctx.enter_context(tc.tile_pool(name="aN_pool", bufs=1))
    ot_pool = ctx.enter_context(tc.tile_pool(name="ot_pool", bufs=3))

    # aoutN stacked as 3 tiles of 128 partitions (heads packed) so the
    # o-projection contracts in 3 chunks of 128 instead of 4 of 96
    aN = [aN_pool.tile([P, TQ], PV_DT, name=f"aN{i}", tag=f"aN{i}")
          for i in range(3)]
    # per-head write segments: (tile, tile_row0, head_row0, nrows)
    _SEG = {0: [(0, 0, 0, 96)],
            1: [(0, 96, 0, 32), (1, 0, 32, 32), (1, 32, 64, 32)],
            2: [(1, 64, 0, 64), (2, 0, 64, 32)],
            3: [(2, 32, 0, 32), (2, 64, 32, 32), (2, 96, 64, 32)]}

    pvs = {}

    def finish_head(h):
        pv0, pv1 = pvs.pop(h)
        aout = aout_pool.tile([VW, TQ], PV_DT, name="aout", tag="aout")
        # denominator row straight from PSUM so the broadcast matmul does
        # not serialize behind the aout eviction
        den1 = tmp_pool.tile([1, TQ], PV_DT, name="den1", tag="den1")
        nc.vector.tensor_copy(den1[:, 0:512], pv0[HD:HD + 1, :])
        nc.vector.tensor_copy(den1[:, 512:1024], pv1[HD:HD + 1, :])
        nc.vector.tensor_copy(aout[:, 0:512], pv0[:])
        nc.vector.tensor_copy(aout[:, 512:1024], pv1[:])
        denB = s_ps.tile([P, TQ], F32, name="denB", tag="s")
        for n in range(2):
            nc.tensor.matmul(denB[:, n * 512:(n + 1) * 512], ones1_t[:],
                             den1[:, n * 512:(n + 1) * 512])
        recB = tmp_pool.tile([HD, TQ], F32, name="recB", tag="tmp")
        nc.vector.reciprocal_approx_fast(out=recB[:], in_=denB[0:HD, :])
        for (ti, tr, hr, nr) in _SEG[h]:
            nc.vector.tensor_mul(aN[ti][tr:tr + nr, :],
                                 aout[hr:hr + nr, :], recB[hr:hr + nr, :])

    LAG = 3
    G = NH_CORE * NKC
    for g in range(G + LAG):
        if g < G:
            h, kc = divmod(g, NKC)
            if not (h == 0 and kc < PRE):
                emit_schunk(h, kc)
        if g >= LAG:
            h2, kc2 = divmod(g - LAG, NKC)
            if kc2 == 0:
                pvs[h2] = (
                    pv_ps.tile([VW, 512], F32, name=f"pv{h2}0", tag="pv"),
                    pv_ps.tile([VW, 512], F32, name=f"pv{h2}1", tag="pv"))
            pv0, pv1 = pvs[h2]
            pt = PTS[(h2, kc2)]
            vl = vst[kc2][:, h2 * VW:(h2 + 1) * VW]
            first, last = (kc2 == 0), (kc2 == NKC - 1)
            nc.tensor.matmul(pv0[:], vl, pt[:, 0:512], start=first, stop=last)
            nc.tensor.matmul(pv1[:], vl, pt[:, 512:1024],
                             start=first, stop=last)
            PTS.pop((h2, kc2))
            if last:
                finish_head(h2)

    # ---- output projection (h-inner accumulation, 2 matmuls per weight) --
    for m in range(MTILES):
        po0 = s_ps.tile([P, 512], F32, name="po0", tag="s")
        po1 = s_ps.tile([P, 512], F32, name="po1", tag="s")
        for i in range(3):
            lhs = wo_t[i][:, m * P:(m + 1) * P]
            nc.tensor.matmul(po0[:], lhs, aN[i][:, 0:512],
                             start=(i == 0), stop=(i == 2))
            nc.tensor.matmul(po1[:], lhs, aN[i][:, 512:1024],
                             start=(i == 0), stop=(i == 2))
        ot = ot_pool.tile([P, TQ], PV_DT, name="ot", tag="ot")
        nc.vector.tensor_copy(ot[:, 0:512], po0[:])
        nc.scalar.copy(ot[:, 512:1024], po1[:])
        eng = nc.sync if m % 2 == 0 else nc.gpsimd
        eng.dma_start(out=oT[m * P:(m + 1) * P, :], in_=ot[:])


# ---------------------------------------------------------------- host side
def _rope_tables(coords, T):
    """Feature-major cos/sin tables [HD, T] with the sign fold.

    Row j < 48 of the rotated output is q[j]*cos_j - q[j+48]*sin_j and row
    j >= 48 is q[j]*cos_{j-48} + q[j-48]*sin_{j-48}; the device computes
    rot = q * cE + swap(q) * sE with swap(q)[j] = q[(j+48) % 96].
    """
    coords = np.asarray(coords, np.float32)
    inv_freq = (1.0 / (ROPE_BASE ** (np.arange(FREQ_PER_AXIS, dtype=np.float32)
                                     / FREQ_PER_AXIS))).astype(np.float32)
    ang = coords[:, :, None] * inv_freq[None, None, :]   # [T, 3, 16]
    ang = ang.reshape(T, ROPE_HALF)                      # [T, 48]
    sin = np.sin(ang).astype(np.float32).T               # [48, T]
    cos = np.cos(ang).astype(np.float32).T
    cE = np.concatenate([cos, cos], axis=0)              # [96, T]
    sE = np.concatenate([-sin, sin], axis=0)
    return (np.ascontiguousarray(cE).astype(QK_NP),
            np.ascontiguousarray(sE).astype(QK_NP))


def _make_in_maps(inputs):
    x = np.asarray(inputs["x"], np.float32)
    memory = np.asarray(inputs["memory"], np.float32)
    qc = np.asarray(inputs["query_coords"], np.float32)
    mc = np.asarray(inputs["memory_coords"], np.float32)
    Wq = np.asarray(inputs["Wq"], np.float32)
    Wk = np.asarray(inputs["Wk"], np.float32)
    Wv = np.asarray(inputs["Wv"], np.float32)
    Wo = np.asarray(inputs["Wo"], np.float32)
    bq = np.asarray(inputs["bq"], np.float32)
    bk = np.asarray(inputs["bk"], np.float32)

    WqT = np.ascontiguousarray(Wq.T).astype(QK_NP)   # [in, out]
    WkT = np.ascontiguousarray(Wk.T).astype(QK_NP)
    WvT = np.ascontiguousarray(Wv.T).astype(PV_NP)
    WoT = np.ascontiguousarray(Wo.T).astype(PV_NP)

    per_batch = []
    for b in range(B):
        cqE, sqE = _rope_tables(qc[b], TQ)
        ckE, skE = _rope_tables(mc[b], TK)
        entry = {
            "xT": np.ascontiguousarray(x[b].T).astype(QK_NP),
            "memT": np.ascontiguousarray(memory[b].T).astype(QK_NP),
            "cqE": cqE, "sqE": sqE, "ckE": ckE, "skE": skE,
        }
        per_batch.append(entry)

    in_maps = []
    for core in range(N_CORES):
        b, g = divmod(core, NH_CORE)
        sl = slice(g * HG, (g + 1) * HG)
        m = dict(per_batch[b])
        m["wqT"] = np.ascontiguousarray(WqT[:, sl])
        m["wkT"] = np.ascontiguousarray(WkT[:, sl])
        m["wvT"] = np.ascontiguousarray(WvT[:, sl])
        m["woT"] = np.ascontiguousarray(WoT[sl, :])
        m["bq4"] = np.ascontiguousarray(bq[sl].reshape(NH_CORE, HD).T)
        m["bk4"] = np.ascontiguousarray(bk[sl].reshape(NH_CORE, HD).T)
        in_maps.append(m)
    return in_maps


def _assemble(results, inputs):
    Wo = np.asarray(inputs["Wo"], np.float32)
    bv = np.asarray(inputs["bv"], np.float32)
    bo = np.asarray(inputs["bo"], np.float32)
    cvec = (bv @ Wo.T + bo).astype(np.float32)   # exact: attn rows sum to 1
    out = np.empty((B, TQ, D), np.float32)
    for b in range(B):
        acc = np.zeros((D, TQ), np.float32)
        for g in range(NH_CORE):
            acc += np.asarray(results[b * NH_CORE + g]["oT"], np.float32)
        out[b] = acc.T + cvec
    return out


_NC_CACHE = None


def _get_nc():
    global _NC_CACHE
    if _NC_CACHE is None:
        _NC_CACHE = _build_nc()
    return _NC_CACHE


_RUNNER = None


def _get_runner():
    """Reusable jitted PJRT executable (same lowering run_bass_kernel_spmd
    uses under axon) so repeated kernel() calls skip recompilation."""
    global _RUNNER
    if _RUNNER is not None:
        return _RUNNER
    import jax
    from jax.sharding import Mesh, PartitionSpec
    try:
        from jax.experimental.shard_map import shard_map
    except ImportError:
        from jax import shard_map
    from concourse import bass2jax

    nc = _get_nc()
    bass2jax.install_neuronx_cc_hook()
    partition_name = (nc.partition_id_tensor.name
                      if nc.partition_id_tensor else None)
    in_names, out_names, out_avals, zero_outs = [], [], [], []
    for alloc in nc.m.functions[0].allocations:
        if not isinstance(alloc, mybir.MemoryLocationSet):
            continue
        name = alloc.memorylocations[0].name
        if alloc.kind == "ExternalInput":
            if name != partition_name:
                in_names.append(name)
        elif alloc.kind == "ExternalOutput":
            out_names.append(name)
            shape = tuple(alloc.tensor_shape)
            dtype = mybir.dt.np(alloc.dtype)
            out_avals.append(jax.core.ShapedArray(shape, dtype))
            zero_outs.append(np.zeros(shape, dtype))
    n_params = len(in_names)
    all_in = list(in_names) + list(out_names)
    if partition_name is not None:
        all_in.append(partition_name)

    def _b(*args):
        operands = list(args)
        if partition_name is not None:
            operands.append(bass2jax.partition_id_tensor())
        return tuple(bass2jax._bass_exec_p.bind(
            *operands, out_avals=tuple(out_avals), in_names=tuple(all_in),
            out_names=tuple(out_names), lowering_input_output_aliases=(),
            sim_require_finite=True, sim_require_nnan=True, nc=nc))

    devices = jax.devices()[:N_CORES]
    mesh = Mesh(np.asarray(devices), ("core",))
    nio = n_params + len(out_avals)
    fn = jax.jit(shard_map(_b, mesh=mesh,
                           in_specs=(PartitionSpec("core"),) * nio,
                           out_specs=(PartitionSpec("core"),) * len(out_avals),
                           check_rep=False), keep_unused=True)

    def run(in_maps):
        per_core = [[np.asarray(m[n]) for n in in_names] for m in in_maps]
        concat_in = [np.concatenate([per_core[c][i] for c in range(N_CORES)],
                                    axis=0) for i in range(n_params)]
        concat_zeros = [np.zeros((N_CORES * z.shape[0], *z.shape[1:]), z.dtype)
                        for z in zero_outs]
        outs = fn(*concat_in, *concat_zeros)
        return [
            {name: np.asarray(outs[i]).reshape(N_CORES, *out_avals[i].shape)[c]
             for i, name in enumerate(out_names)}
            for c in range(N_CORES)
        ]

    _RUNNER = run
    return run


_CALLED = False


def kernel(**inputs) -> np.ndarray:
    """Full-input entry point: shards across 8 NeuronCores, runs the Bass
    kernel, gathers and unshards. First call uses run_bass_kernel_spmd
    (compile + run); later calls reuse the cached executable."""
    global _CALLED
    in_maps = _make_in_maps(inputs)
    if not _CALLED:
        _CALLED = True
        nc = _get_nc()
        res = run_bass_kernel_spmd(nc, in_maps, list(range(N_CORES)))
        results = res.results
    else:
        results = _get_runner()(in_maps)
    return _assemble(results, inputs)
